# revision 13
# baseline (speedup 1.0000x reference)
"""Fused single-launch Trainium2 kernel for nn_Attention_39565238731193.

Per core (2048 pixels): qkv GEMMs (TensorE, bf16) -> per-pixel two-stage
attention (DVE/ACT/GPSIMD with stride-0 broadcast APs) -> channel scramble
(DMA through DRAM) -> proj GEMM + bias -> output rows in r'-order, fixed up
on host.

Math reformulation vs the reference (all exact up to fp rounding):
  attn_head softmax needs no max-subtraction (|G-hat| <= 1); the q/k head
  mixing  q~ = A qn, k~ = A kn  collapses into  S = k^T B~ q  with
  B~ = diag(rk) (A^T A) diag(rq), so only one 8x8 per-pixel matrix reaches
  the big stage-2 contractions. Stage-2 softmax normalizer folds into
  v~ = v * (1/Z_d) and the final v+v doubling is folded into Wv on host.
"""
import sys
import time

sys.path.insert(0, "/opt/trn_rl_repo")

import numpy as np
import ml_dtypes
from contextlib import ExitStack

from concourse import bass
import concourse.mybir as mybir
from concourse.bass_utils import run_bass_kernel_spmd

F32 = mybir.dt.float32
BF16 = mybir.dt.bfloat16
AF = mybir.ActivationFunctionType
MUL = mybir.AluOpType.mult
ADD = mybir.AluOpType.add
AX = mybir.AxisListType.X

NCORES = 8
C = 512
H, D = 8, 64

LAST_EXEC_NS = None
LAST_WALL_NS = None
_CACHE = {}


class _Prog:
    """Static program with automatic RAW/WAR/WAW semaphore insertion.

    Ops are emitted per-engine in list order. Each op declares the buffers it
    reads/writes; dependencies become wait_ge on the producing engine's
    semaphore (every engine's ops inc its own semaphore: compute +1, DMA +16).
    """

    BIG = 1 << 60

    def __init__(self):
        self.ops = []  # dict: engine, fn, deps(set of op ids)
        self.writes = {}  # tensor -> list of (op id, lo, hi)
        self.reads = {}   # tensor -> list of (op id, lo, hi)

    @staticmethod
    def _key(b):
        # "name" (whole tensor) or ("name", lo, hi) element ranges
        if isinstance(b, str):
            return b, 0, _Prog.BIG
        return b

    def op(self, engine, fn, reads=(), writes=(), sem=None):
        deps = set()
        rds = [self._key(b) for b in reads]
        wrs = [self._key(b) for b in writes]
        for name, lo, hi in rds:
            for oid, wlo, whi in self.writes.get(name, ()):
                if wlo < hi and lo < whi:
                    deps.add(oid)
        for name, lo, hi in wrs:
            for oid, rlo, rhi in self.reads.get(name, ()):
                if rlo < hi and lo < rhi:
                    deps.add(oid)
            for oid, wlo, whi in self.writes.get(name, ()):
                if wlo < hi and lo < whi:
                    deps.add(oid)
        i = len(self.ops)
        self.ops.append({"engine": engine, "sem": sem or engine, "fn": fn, "deps": deps})
        for name, lo, hi in rds:
            self.reads.setdefault(name, []).append((i, lo, hi))
        for name, lo, hi in wrs:
            # drop fully-covered older entries to bound list growth
            self.writes[name] = [
                t for t in self.writes.get(name, ()) if not (lo <= t[1] and t[2] <= hi)
            ] + [(i, lo, hi)]
            self.reads[name] = [
                t for t in self.reads.get(name, ()) if not (lo <= t[1] and t[2] <= hi)
            ]
        return i

    def emit(self, nc, sems):
        # assign per-op completion value on its engine's semaphore
        cum = {e: 0 for e in sems}
        val = [0] * len(self.ops)
        for i, o in enumerate(self.ops):
            e = o["engine"]
            cum[e] += 16 if e == "s" else 1
            val[i] = cum[e]
        by_engine = {e: [] for e in sems}
        for i, o in enumerate(self.ops):
            by_engine[o["engine"]].append(i)

        def run_engine(ename, eng):
            watermark = {}
            for i in by_engine[ename]:
                o = self.ops[i]
                # coalesce: one wait per dep engine at max needed value
                need = {}
                for d in o["deps"]:
                    de = self.ops[d]["engine"]
                    if de == ename and val[d] <= watermark.get(de, 0):
                        pass
                    need[de] = max(need.get(de, 0), val[d])
                for de, v in sorted(need.items()):
                    if watermark.get(de, 0) >= v:
                        continue
                    eng.wait_ge(sems[de], v)
                    watermark[de] = v
                last = o["fn"](eng)
                last.then_inc(sems[ename], 16 if ename == "s" else 1)
                # own op raises own watermark implicitly? no: own sem value
                # only advances when the instruction completes; later same-
                # engine ops that depend on it still need an explicit wait.

        return run_engine


def build_fused(npx):
    G = npx // 128
    nc = bass.Bass()
    xall = nc.dram_tensor("xall", [128, 4 * npx], BF16, kind="ExternalInput")
    wall = nc.dram_tensor("wall", [128, 16 * 512], BF16, kind="ExternalInput")
    biasin = nc.dram_tensor("biasin", [1, 512], BF16, kind="ExternalInput")
    yout = nc.dram_tensor("yout", [npx, 512], BF16, kind="ExternalOutput")
    scr = nc.dram_tensor("scr", [npx, 512], BF16, kind="Internal")

    with ExitStack() as ctx:
        def sb(name, shp, dt):
            return ctx.enter_context(nc.sbuf_tensor(name, shp, dt))

        xs = sb("xs", [128, 4 * npx], BF16)
        ws = sb("ws", [128, 16 * 512], BF16)
        bias = sb("bias", [1, 512], BF16)
        ones = sb("ones", [1, 128], BF16)
        qbT2 = [sb(f"qbTb{i}", [128, 512], BF16) for i in range(2)]  # [e*8+h]
        kbT2 = [sb(f"kbTb{i}", [128, 512], BF16) for i in range(2)]  # [d*8+h]
        vb2 = [sb(f"vbb{i}", [128, 512], BF16) for i in range(2)]    # [h*64+d]
        zsq = sb("zsq", [128, 1024], BF16)  # q,k squares
        zmid = sb("zmid", [128, 4096], BF16)
        zmid2 = sb("zmid2", [128, 16384], BF16)
        zbig = sb("zbig", [128, 32768], BF16)
        Sf = sb("Sf", [128, 4096], F32)
        est2 = [sb(f"estb{i}", [128, 4096], BF16) for i in range(2)]
        nrm = sb("nrm", [128, 16], F32)     # nq | nk
        rcp = sb("rcp", [128, 16], F32)     # tq | tk
        rqk2 = [sb(f"rqkb{i}", [128, 16], F32) for i in range(2)]
        G64 = sb("G64", [128, 64], F32)
        tv8 = sb("tv8", [128, 8], F32)
        rv8 = sb("rv8", [128, 8], F32)
        rvv = sb("rvv", [128, 64], F32)
        gh = sb("gh", [128, 64], F32)
        eg = sb("eg", [128, 64], F32)
        sa8 = sb("sa8", [128, 8], F32)
        tt8 = sb("tt8", [128, 8], F32)
        w8 = sb("w8", [128, 8], F32)
        t1 = sb("t1", [128, 64], F32)
        zb = sb("zb", [128, 512], F32)
        B64 = sb("B64", [128, 64], F32)
        rkq = sb("rkq", [128, 64], F32)
        btT = sb("btT", [128, 64], BF16)    # [g*8+h]
        k2f = sb("k2f", [128, 512], F32)    # [g*64+d]
        k2T2 = [sb(f"k2Tb{i}", [128, 512], BF16) for i in range(2)]  # [d*8+g]
        Z64 = sb("Z64", [128, 64], F32)
        rz = sb("rz", [128, 64], BF16)
        vt2 = [sb(f"vtb{i}", [128, 512], BF16) for i in range(2)]
        oab2 = [sb(f"oabb{i}", [128, 512], BF16) for i in range(2)]
        xscr = sb("xscr", [128, 4 * npx], BF16)
        yb = sb("yb", [128, 512], BF16)

        psq2 = [ctx.enter_context(nc.psum_tensor(f"psqb{i}", [128, 512], F32)) for i in range(2)]
        psk2 = [ctx.enter_context(nc.psum_tensor(f"pskb{i}", [128, 512], F32)) for i in range(2)]
        psv2 = [ctx.enter_context(nc.psum_tensor(f"psvb{i}", [128, 512], F32)) for i in range(2)]
        psy = ctx.enter_context(nc.psum_tensor("psy", [128, 512], F32))

        sems = {
            k: ctx.enter_context(nc.semaphore(f"sem_{k}"))
            for k in ("t", "a", "v", "p", "si", "sc", "sg", "sy")
        }

        P = _Prog()
        xsr = xs[:].rearrange("p (c n) -> p c n", c=4)
        wsr = ws[:].rearrange("p (w c n) -> p w c n", w=4, c=4)

        # ---- loads ----
        P.op("s", lambda e: e.dma_start(out=xs[:], in_=xall[:]), writes=["xs"], sem="si")
        P.op("s", lambda e: e.dma_start(out=ws[:], in_=wall[:]), writes=["ws"], sem="si")
        P.op("s", lambda e: e.dma_start(out=bias[:], in_=biasin[:]), writes=["bias"], sem="si")
        P.op("p", lambda e: e.memset(ones[:], 1.0), writes=["ones"])

        def mm_qkv(g, wi, ps, psname):
            def fn(e):
                for ci in range(4):
                    mm = e.matmul(
                        ps[:],
                        xsr[:, ci, g * 128 : (g + 1) * 128],
                        wsr[:, wi, ci, :],
                        start=(ci == 0),
                        stop=(ci == 3),
                    )
                return mm
            return fn

        # ---- per-group attention ----
        def emit_group(g):
            par = g % 2
            qbT, kbT, vb = qbT2[par], kbT2[par], vb2[par]
            est, rqk, k2T = est2[par], rqk2[par], k2T2[par]
            vt, oab = vt2[par], oab2[par]
            psq, psk, psv = psq2[par], psk2[par], psv2[par]
            pfx = f"_{par}"
            P.op("t", mm_qkv(g, 0, psq, "psq"), reads=["xs", "ws"], writes=["psq" + pfx])
            P.op("t", mm_qkv(g, 1, psk, "psk"), reads=["xs", "ws"], writes=["psk" + pfx])
            P.op("t", mm_qkv(g, 2, psv, "psv"), reads=["xs", "ws"], writes=["psv" + pfx])

            # psum -> sbuf copies (ACT), with layout transforms
            P.op(
                "a",
                lambda e: e.activation(
                    qbT[:].rearrange("p (e h) -> p h e", h=H), psq[:], AF.Copy
                ),
                reads=["psq" + pfx],
                writes=["qbT" + pfx],
            )
            P.op(
                "a",
                lambda e: e.activation(
                    kbT[:].rearrange("p (d h) -> p h d", h=H), psk[:], AF.Copy
                ),
                reads=["psk" + pfx],
                writes=["kbT" + pfx],
            )
            P.op(
                "a",
                lambda e: e.activation(vb[:], psv[:], AF.Copy),
                reads=["psv" + pfx],
                writes=["vb" + pfx],
            )

            # squares for q,k norms (gpsimd)
            P.op(
                "p",
                lambda e: e.tensor_tensor(zsq[:, 0:512], qbT[:], qbT[:], op=MUL),
                reads=["qbT" + pfx],
                writes=["zsq_q"],
            )
            P.op(
                "p",
                lambda e: e.tensor_tensor(zsq[:, 512:1024], kbT[:], kbT[:], op=MUL),
                reads=["kbT" + pfx],
                writes=["zsq_k"],
            )
            # norm sums: nq over e for each h; zsq_q layout [e*8+h]
            P.op(
                "v",
                lambda e: e.reduce_sum(
                    nrm[:, 0:8],
                    zsq[:, 0:512].rearrange("p (e h) -> p h e", h=H),
                    axis=AX,
                ),
                reads=["zsq_q"],
                writes=["nq"],
            )
            P.op(
                "v",
                lambda e: e.reduce_sum(
                    nrm[:, 8:16],
                    zsq[:, 512:1024].rearrange("p (d h) -> p h d", h=H),
                    axis=AX,
                ),
                reads=["zsq_k"],
                writes=["nk"],
            )
            P.op(
                "v",
                lambda e: e.reciprocal(rcp[:], nrm[:]),
                reads=["nq", "nk"],
                writes=["rcp"],
            )
            P.op(
                "a",
                lambda e: e.activation(rqk[:], rcp[:], AF.Sqrt),
                reads=["rcp"],
                writes=["rqk" + pfx],
            )

            # gram of v: zmid[h,g,d] = vb[h,d]*vb[g,d]
            def gram(e):
                v3 = vb[:].rearrange("p (h d) -> p h d", h=H)
                in0 = v3.unsqueeze(2).broadcast_to([128, H, H, D])
                in1 = v3.unsqueeze(1).broadcast_to([128, H, H, D])
                return e.tensor_tensor(
                    zmid[:].rearrange("p (h g d) -> p h g d", h=H, g=H),
                    in0, in1, op=MUL,
                )
            P.op("v", gram, reads=["vb" + pfx], writes=["zmid"])
            P.op(
                "v",
                lambda e: e.reduce_sum(
                    G64[:],
                    zmid[:].rearrange("p (hg d) -> p hg d", d=D),
                    axis=AX,
                ),
                reads=["zmid"],
                writes=["G64"],
            )
            P.op(
                "v",
                lambda e: e.reciprocal(tv8[:], G64[:, 0 : 64 : H + 1]),
                reads=["G64"],
                writes=["tv8"],
            )
            P.op(
                "a",
                lambda e: e.activation(rv8[:], tv8[:], AF.Sqrt),
                reads=["tv8"],
                writes=["rv8"],
            )
            P.op(
                "v",
                lambda e: e.tensor_tensor(
                    rvv[:].rearrange("p (h g) -> p h g", h=H),
                    rv8[:].unsqueeze(2).broadcast_to([128, H, H]),
                    rv8[:].unsqueeze(1).broadcast_to([128, H, H]),
                    op=MUL,
                ),
                reads=["rv8"],
                writes=["rvv"],
            )
            P.op(
                "v",
                lambda e: e.tensor_tensor(gh[:], G64[:], rvv[:], op=MUL),
                reads=["G64", "rvv"],
                writes=["gh"],
            )
            P.op(
                "a",
                lambda e: e.activation(eg[:], gh[:], AF.Exp),
                reads=["gh"],
                writes=["eg"],
            )
            P.op(
                "v",
                lambda e: e.reduce_sum(
                    sa8[:], eg[:].rearrange("p (f g) -> p f g", f=H), axis=AX
                ),
                reads=["eg"],
                writes=["sa8"],
            )
            P.op(
                "v",
                lambda e: e.reciprocal(tt8[:], sa8[:]),
                reads=["sa8"],
                writes=["tt8"],
            )
            P.op(
                "a",
                lambda e: e.activation(w8[:], tt8[:], AF.Square),
                reads=["tt8"],
                writes=["w8"],
            )
            P.op(
                "v",
                lambda e: e.tensor_tensor(
                    t1[:].rearrange("p (f g) -> p f g", f=H),
                    eg[:].rearrange("p (f g) -> p f g", f=H),
                    w8[:].unsqueeze(2).broadcast_to([128, H, H]),
                    op=MUL,
                ),
                reads=["eg", "w8"],
                writes=["t1"],
            )
            # zb[h,g,f] = t1[f,h] * eg[f,g]
            def zbmul(e):
                t1v = t1[:].rearrange("p (f h) -> p f h", f=H)  # [p,f,h]
                egv = eg[:].rearrange("p (f g) -> p f g", f=H)
                in0 = t1v.transpose([0, 2, 1]).unsqueeze(2).broadcast_to([128, H, H, H])
                in1 = egv.transpose([0, 2, 1]).unsqueeze(1).broadcast_to([128, H, H, H])
                return e.tensor_tensor(
                    zb[:].rearrange("p (h g f) -> p h g f", h=H, g=H), in0, in1, op=MUL
                )
            P.op("v", zbmul, reads=["t1", "eg"], writes=["zb"])
            P.op(
                "v",
                lambda e: e.reduce_sum(
                    B64[:], zb[:].rearrange("p (hg f) -> p hg f", f=H), axis=AX
                ),
                reads=["zb"],
                writes=["B64"],
            )
            # rkq[h,g] = rk[h]*rq[g]
            P.op(
                "v",
                lambda e: e.tensor_tensor(
                    rkq[:].rearrange("p (h g) -> p h g", h=H),
                    rqk[:, 8:16].unsqueeze(2).broadcast_to([128, H, H]),
                    rqk[:, 0:8].unsqueeze(1).broadcast_to([128, H, H]),
                    op=MUL,
                ),
                reads=["rqk" + pfx],
                writes=["rkq"],
            )
            # btT[g*8+h] = B[h,g] * rkq[h,g]
            P.op(
                "v",
                lambda e: e.scalar_tensor_tensor(
                    btT[:].rearrange("p (g h) -> p h g", g=H),
                    B64[:], 1.0, rkq[:], op0=MUL, op1=MUL,
                ),
                reads=["B64", "rkq"],
                writes=["btT"],
            )
            # zk2[g,d,h] = btT[g,h] * kbT[d,h]
            def zk2mul(e):
                b3 = btT[:].rearrange("p (g h) -> p g h", g=H)
                k3 = kbT[:].rearrange("p (d h) -> p d h", d=D)
                in0 = b3.unsqueeze(2).broadcast_to([128, H, D, H])
                in1 = k3.unsqueeze(1).broadcast_to([128, H, D, H])
                return e.tensor_tensor(
                    zmid[:].rearrange("p (g d h) -> p g d h", g=H, d=D),
                    in0, in1, op=MUL,
                )
            P.op("p", zk2mul, reads=["btT", "kbT" + pfx], writes=["zmid"])
            P.op(
                "v",
                lambda e: e.reduce_sum(
                    k2f[:], zmid[:].rearrange("p (gd h) -> p gd h", h=H), axis=AX
                ),
                reads=["zmid"],
                writes=["k2f"],
            )
            # k2T[d*8+g] = k2f[g*64+d] (cast+transpose)
            P.op(
                "a",
                lambda e: e.activation(
                    k2T[:].rearrange("p (d g) -> p g d", d=D),
                    k2f[:], AF.Copy,
                ),
                reads=["k2f"],
                writes=["k2T" + pfx],
            )
            # zS[d,e,g] = k2T[d,g] * qbT[e,g]  -- split by d across V (0:36) / P (36:64)
            DV = 36  # V's d-share
            def zsmul_part(d0, d1):
                def fn(e):
                    k3 = k2T[:].rearrange("p (d g) -> p d g", d=D)[:, d0:d1, :]
                    q3 = qbT[:].rearrange("p (e g) -> p e g", e=D)
                    nd = d1 - d0
                    in0 = k3.unsqueeze(2).broadcast_to([128, nd, D, H])
                    in1 = q3.unsqueeze(1).broadcast_to([128, nd, D, H])
                    out = zbig[:, d0 * 512 : d1 * 512].rearrange(
                        "p (d e g) -> p d e g", e=D, g=H
                    )
                    return e.tensor_tensor(out, in0, in1, op=MUL)
                return fn
            P.op("v", zsmul_part(0, DV), reads=["k2T" + pfx, "qbT" + pfx], writes=[("zbig", 0, 18432)])
            P.op("p", zsmul_part(DV, D), reads=["k2T" + pfx, "qbT" + pfx], writes=[("zbig", 18432, 32768)])

            # S-tree: half-pair adds over g (unit-stride => DVE 2x), V/P parts
            def hp(src_ap, dst_ap, k):
                # src viewed [p, x, k] -> dst [p, x, k//2] adding halves
                def fn(e):
                    s3 = src_ap.rearrange("p (x k) -> p x k", k=k)
                    d3 = dst_ap.rearrange("p (x k) -> p x k", k=k // 2)
                    return e.tensor_tensor(
                        d3, s3[:, :, 0 : k // 2], s3[:, :, k // 2 : k], op=ADD
                    )
                return fn
            DE1 = DV * 64  # 2304
            P.op("v", hp(zbig[:, 0 : DE1 * 8], zmid2[:, 0 : DE1 * 4], 8),
                 reads=[("zbig", 0, 18432)], writes=[("zmid2", 0, 9216)])
            P.op("p", hp(zbig[:, DE1 * 8 : 32768], zmid2[:, DE1 * 4 : 16384], 8),
                 reads=[("zbig", 18432, 32768)], writes=[("zmid2", 9216, 16384)])
            P.op("v", hp(zmid2[:, 0 : DE1 * 4], zbig[:, 0 : DE1 * 2], 4),
                 reads=[("zmid2", 0, 9216)], writes=[("zbig", 0, 4608)])
            P.op("p", hp(zmid2[:, DE1 * 4 : 16384], zbig[:, DE1 * 2 : 8192], 4),
                 reads=[("zmid2", 9216, 16384)], writes=[("zbig", 4608, 8192)])
            P.op("v", hp(zbig[:, 0 : DE1 * 2], Sf[:, 0:DE1], 2),
                 reads=[("zbig", 0, 4608)], writes=[("Sf", 0, 2304)])
            P.op("p", hp(zbig[:, DE1 * 2 : 8192], Sf[:, DE1:4096], 2),
                 reads=[("zbig", 4608, 8192)], writes=[("Sf", 2304, 4096)])

            # expS transposed: est[e*64+d] = exp(S[d*64+e])
            P.op(
                "a",
                lambda e: e.activation(
                    est[:].rearrange("p (e d) -> p d e", e=D), Sf[:], AF.Exp
                ),
                reads=[("Sf", 0, 4096)],
                writes=["est" + pfx],
            )
            # Z[d] = sum_e expS[d,e]
            P.op(
                "v",
                lambda e: e.reduce_sum(
                    Z64[:], est[:].rearrange("p (e d) -> p d e", e=D), axis=AX
                ),
                reads=["est" + pfx],
                writes=["Z64"],
            )
            def rzrecip(e):
                with nc.allow_low_precision(reason="1/Z at bf16 is within budget"):
                    return e.reciprocal(rz[:], Z64[:])
            P.op("v", rzrecip, reads=["Z64"], writes=["rz"])
            # vt[h,d] = vb[h,d]*rz[d]
            P.op(
                "v",
                lambda e: e.tensor_tensor(
                    vt[:].rearrange("p (h d) -> p h d", h=H),
                    vb[:].rearrange("p (h d) -> p h d", h=H),
                    rz[:].unsqueeze(1).broadcast_to([128, H, D]),
                    op=MUL,
                ),
                reads=["vb" + pfx, "rz"],
                writes=["vt" + pfx],
            )
            # zO[h,e,d] = vt[h,d]*est[e,d]  -- split by h across V (0:5) / P (5:8)
            HV = 5
            def zomul_part(h0, h1):
                def fn(e):
                    v3 = vt[:].rearrange("p (h d) -> p h d", h=H)[:, h0:h1, :]
                    e3 = est[:].rearrange("p (e d) -> p e d", e=D)
                    nh = h1 - h0
                    in0 = v3.unsqueeze(2).broadcast_to([128, nh, D, D])
                    in1 = e3.unsqueeze(1).broadcast_to([128, nh, D, D])
                    out = zbig[:, h0 * 4096 : h1 * 4096].rearrange(
                        "p (h e d) -> p h e d", e=D, d=D
                    )
                    return e.tensor_tensor(out, in0, in1, op=MUL)
                return fn
            P.op("v", zomul_part(0, HV),
                 reads=["vt" + pfx, "est" + pfx], writes=[("zbig", 0, 20480)])
            P.op("p", zomul_part(HV, H),
                 reads=["vt" + pfx, "est" + pfx], writes=[("zbig", 20480, 32768)])

            # O-tree: half-pair adds over d, V/P parts (he-aligned)
            HE1 = HV * 64  # 320
            P.op("v", hp(zbig[:, 0 : HE1 * 64], zmid2[:, 0 : HE1 * 32], 64),
                 reads=[("zbig", 0, 20480)], writes=[("zmid2", 0, 10240)])
            P.op("p", hp(zbig[:, HE1 * 64 : 32768], zmid2[:, HE1 * 32 : 16384], 64),
                 reads=[("zbig", 20480, 32768)], writes=[("zmid2", 10240, 16384)])
            P.op("v", hp(zmid2[:, 0 : HE1 * 32], zbig[:, 0 : HE1 * 16], 32),
                 reads=[("zmid2", 0, 10240)], writes=[("zbig", 0, 5120)])
            P.op("p", hp(zmid2[:, HE1 * 32 : 16384], zbig[:, HE1 * 16 : 8192], 32),
                 reads=[("zmid2", 10240, 16384)], writes=[("zbig", 5120, 8192)])
            P.op("v", hp(zbig[:, 0 : HE1 * 16], zmid[:, 0 : HE1 * 8], 16),
                 reads=[("zbig", 0, 5120)], writes=[("zmid", 0, 2560)])
            P.op("p", hp(zbig[:, HE1 * 16 : 8192], zmid[:, HE1 * 8 : 4096], 16),
                 reads=[("zbig", 5120, 8192)], writes=[("zmid", 2560, 4096)])
            P.op("v", hp(zmid[:, 0 : HE1 * 8], zmid2[:, 0 : HE1 * 4], 8),
                 reads=[("zmid", 0, 2560)], writes=[("zmid2", 0, 1280)])
            P.op("p", hp(zmid[:, HE1 * 8 : 4096], zmid2[:, HE1 * 4 : 2048], 8),
                 reads=[("zmid", 2560, 4096)], writes=[("zmid2", 1280, 2048)])
            P.op("v", hp(zmid2[:, 0 : HE1 * 4], zmid[:, 0 : HE1 * 2], 4),
                 reads=[("zmid2", 0, 1280)], writes=[("zmid", 0, 640)])
            P.op("p", hp(zmid2[:, HE1 * 4 : 2048], zmid[:, HE1 * 2 : 1024], 4),
                 reads=[("zmid2", 1280, 2048)], writes=[("zmid", 640, 1024)])
            P.op("v", hp(zmid[:, 0 : HE1 * 2], oab[:, 0:HE1], 2),
                 reads=[("zmid", 0, 640)], writes=[("oab" + pfx, 0, 320)])
            P.op("p", hp(zmid[:, HE1 * 2 : 1024], oab[:, HE1:512], 2),
                 reads=[("zmid", 640, 1024)], writes=[("oab" + pfx, 320, 512)])

            # store attention output rows to DRAM scratch
            def store_scr(g):
                def fn(e):
                    return e.dma_start(
                        out=scr[g * 128 : (g + 1) * 128, :], in_=oab[:]
                    )
                return fn
            P.op("s", store_scr(g), reads=["oab" + pfx], writes=[f"scr{g}", "scr_order"], sem="sc")

        for g in range(G):
            emit_group(g)

        # ---- scramble gather: xscr blocks ----
        # xscrT[c'=pm*8+h, r'=pb*64+e] = scr[p=pb*64+pm, h*64+e]
        def gather(e):
            src_r = scr[:].rearrange("(pb pm) (h e) -> pm pb h e", pm=64, h=H)
            last = None
            for Bi in range(4):
                xb = xscr[:, Bi * npx : (Bi + 1) * npx]
                for pmq in range(16):
                    pm = 16 * Bi + pmq
                    dst = xb[8 * pmq : 8 * pmq + 8, :].rearrange(
                        "h (pb e) -> h pb e", e=D
                    )
                    s_ap = src_r[pm, :, :, :].transpose([1, 0, 2])
                    last = e.dma_start(out=dst, in_=s_ap)
            return last
        gid = P.op(
            "s", gather,
            reads=[f"scr{g}" for g in range(G)],
            writes=["xscr"],
            sem="sg",
        )
        # gather emits 64 DMAs but op framework incs once; fix: account below.
        _GATHER_DMAS = 64

        # ---- proj ----
        for rg in range(G):
            def mm_proj(rg):
                def fn(e):
                    for ci in range(4):
                        e.matmul(
                            psy[:],
                            xscr[:, ci * npx + rg * 128 : ci * npx + (rg + 1) * 128],
                            wsr[:, 3, ci, :],
                            start=(ci == 0),
                            stop=False,
                        )
                    mm = e.matmul(
                        psy[:], ones[:], bias[:], start=False, stop=True
                    )
                    return mm
                return fn
            P.op("t", mm_proj(rg), reads=["xscr", "ws", "ones", "bias"],
                 writes=["psy"])
            P.op(
                "a",
                lambda e: e.activation(yb[:], psy[:], AF.Copy),
                reads=["psy"],
                writes=["yb"],
            )
            def store_y(rg):
                def fn(e):
                    return e.dma_start(
                        out=yout[rg * 128 : (rg + 1) * 128, :], in_=yb[:]
                    )
                return fn
            P.op("s", store_y(rg), reads=["yb"], writes=[f"yout{rg}"], sem="sy")

        # ---- emit ----
        # fix gather op inc accounting: it emits 64 dma_starts, each must inc;
        # we gave it one inc. Simplest: make each dma in gather inc and adjust
        # the cumulative count. Easier: treat gather as 64 increments.
        run_engine = _emit_prog(P, nc, sems, gid, _GATHER_DMAS)

        with nc.Block() as block:
            @block.sync
            def _(eng):
                run_engine("s", eng)

            @block.tensor
            def _(eng):
                run_engine("t", eng)

            @block.scalar
            def _(eng):
                run_engine("a", eng)

            @block.vector
            def _(eng):
                run_engine("v", eng)

            @block.gpsimd
            def _(eng):
                run_engine("p", eng)

    return nc


DMA_SEMS = ("si", "sc", "sg", "sy")
# sems whose waiters must always wait for the sem's running total at that
# point (their DMAs complete out of order):
TOTAL_SEMS = ("si", "sg")


def _emit_prog(P, nc, sems, gather_id, gather_n):
    """Emit P's ops; the gather op emits gather_n DMAs, each inc'ing by 16."""
    cum = {k: 0 for k in sems}
    val = [0] * len(P.ops)
    for i, o in enumerate(P.ops):
        k = o["sem"]
        if i == gather_id:
            cum[k] += 16 * gather_n
        else:
            cum[k] += 16 if k in DMA_SEMS else 1
        val[i] = cum[k]
    by_engine = {}
    for i, o in enumerate(P.ops):
        by_engine.setdefault(o["engine"], []).append(i)

    def run_engine(ename, eng):
        watermark = {}
        for i in by_engine.get(ename, ()):
            o = P.ops[i]
            need = {}
            for d in o["deps"]:
                dk = P.ops[d]["sem"]
                v = cum[dk] if dk in TOTAL_SEMS else val[d]
                need[dk] = max(need.get(dk, 0), v)
            for dk, v in sorted(need.items()):
                if watermark.get(dk, 0) >= v:
                    continue
                eng.wait_ge(sems[dk], v)
                watermark[dk] = v
            k = o["sem"]
            if i == gather_id:
                collected = []
                orig = eng.dma_start

                def wrapped(*a, **kw):
                    ins = orig(*a, **kw)
                    collected.append(ins)
                    return ins

                eng.dma_start = wrapped
                try:
                    o["fn"](eng)
                finally:
                    eng.dma_start = orig
                for ins in collected:
                    ins.then_inc(sems[k], 16)
            else:
                last = o["fn"](eng)
                last.then_inc(sems[k], 16 if k in DMA_SEMS else 1)

    return run_engine


def _build_warmup(npx):
    """Trivial kernel with the fused kernel's exact I/O shapes; run once to
    warm the axon/PJRT/jit infrastructure before the timed launch."""
    nc = bass.Bass()
    xall = nc.dram_tensor("xall", [128, 4 * npx], BF16, kind="ExternalInput")
    wall = nc.dram_tensor("wall", [128, 16 * 512], BF16, kind="ExternalInput")
    biasin = nc.dram_tensor("biasin", [1, 512], BF16, kind="ExternalInput")
    yout = nc.dram_tensor("yout", [npx, 512], BF16, kind="ExternalOutput")
    with nc.sbuf_tensor("t", [128, 512], BF16) as t, nc.semaphore("s") as s, nc.Block() as block:
        @block.sync
        def _(sync):
            sync.dma_start(out=t[:], in_=xall[:, 0:512]).then_inc(s, 16)
            sync.wait_ge(s, 16)
            for r in range(npx // 128):
                sync.dma_start(
                    out=yout[r * 128 : (r + 1) * 128, :], in_=t[:]
                ).then_inc(s, 16)
            sync.wait_ge(s, 16 * (1 + npx // 128))
    return nc


def _pack_x(Xc):
    """[npx, 512] -> [128, 4*npx] bf16: out[p, ci, px] = X[px, ci*128+p]"""
    npx = Xc.shape[0]
    xt = Xc.T.reshape(4, 128, npx).transpose(1, 0, 2).reshape(128, 4 * npx)
    return np.ascontiguousarray(xt.astype(ml_dtypes.bfloat16))


def _pack_w(Wq, Wk, Wv2, Wp):
    """4x [512,512] -> [128, 4w*4ci*512co] bf16: out[p,w,ci,co]=W[co,ci*128+p]"""
    ws = np.stack(
        [W.T.reshape(4, 128, 512).transpose(1, 0, 2) for W in (Wq, Wk, Wv2, Wp)],
        axis=1,
    )  # [128, 4w, 4ci, 512]
    return np.ascontiguousarray(
        ws.reshape(128, 16 * 512).astype(ml_dtypes.bfloat16)
    )


def _can_trace():
    """exec_time_ns needs the axon NTFF hook; probing avoids a crash when the
    antenv build lacks it."""
    try:
        from antenv.axon_hooks import get_axon_ntff_profile_hook
    except Exception:
        return False
    try:
        return get_axon_ntff_profile_hook() is not None
    except Exception:
        return False


def kernel(x, Wq, Wk, Wv, conv_w, proj_w, proj_b):
    global LAST_EXEC_NS, LAST_WALL_NS
    x = np.asarray(x, np.float32)
    b, h, w, c = x.shape
    n = h * w
    N = b * n
    npx = N // NCORES
    X = x.reshape(N, c)

    if "fused" not in _CACHE:
        _CACHE["fused"] = build_fused(npx)
        _CACHE["warm"] = _build_warmup(npx)
    nc = _CACHE["fused"]

    wallp = _pack_w(
        np.asarray(Wq, np.float32),
        np.asarray(Wk, np.float32),
        2.0 * np.asarray(Wv, np.float32),
        np.asarray(proj_w, np.float32),
    )
    biasp = np.ascontiguousarray(
        np.asarray(proj_b, np.float32).reshape(1, 512).astype(ml_dtypes.bfloat16)
    )
    in_maps = [
        {
            "xall": _pack_x(X[j * npx : (j + 1) * npx]),
            "wall": wallp,
            "biasin": biasp,
        }
        for j in range(NCORES)
    ]

    # one-time infra warmup (axon/PJRT/jit init), not part of the kernel run
    if "warmed" not in _CACHE:
        win = [
            {
                "xall": np.zeros((128, 4 * npx), ml_dtypes.bfloat16),
                "wall": np.zeros((128, 16 * 512), ml_dtypes.bfloat16),
                "biasin": np.zeros((1, 512), ml_dtypes.bfloat16),
            }
            for _ in range(NCORES)
        ]
        run_bass_kernel_spmd(_CACHE["warm"], win, list(range(NCORES)))
        _CACHE["warmed"] = True

    t0 = time.perf_counter_ns()
    res = None
    if _can_trace():
        try:
            res = run_bass_kernel_spmd(
                nc, in_maps, list(range(NCORES)), trace=True
            )
        except Exception:
            res = None
    if res is None:
        t0 = time.perf_counter_ns()
        res = run_bass_kernel_spmd(nc, in_maps, list(range(NCORES)))
    wall_ns = time.perf_counter_ns() - t0

    LAST_EXEC_NS = res.exec_time_ns
    LAST_WALL_NS = wall_ns

    # unscramble: core j row r' -> y[b=j//2, n' = e*64 + (j%2)*32 + pb]
    y = np.zeros((b, n, c), np.float32)
    rp = np.arange(npx)
    pb2 = rp // 64
    e = rp % 64
    for j in range(NCORES):
        yc = res.results[j]["yout"].astype(np.float32)
        nprime = e * 64 + (j % 2) * 32 + pb2
        y[j // 2, nprime] = yc
    return y.reshape(b, h, w, c)


# revision 15
# speedup vs baseline: 1.0559x; 1.0559x over previous
"""Fused single-launch Trainium2 kernel for nn_Attention_39565238731193.

Per core (2048 pixels): qkv GEMMs (TensorE, bf16) -> per-pixel two-stage
attention (DVE/ACT/GPSIMD with stride-0 broadcast APs) -> channel scramble
(DMA through DRAM) -> proj GEMM + bias -> output rows in r'-order, fixed up
on host.

Math reformulation vs the reference (all exact up to fp rounding):
  attn_head softmax needs no max-subtraction (|G-hat| <= 1); the q/k head
  mixing  q~ = A qn, k~ = A kn  collapses into  S = k^T B~ q  with
  B~ = diag(rk) (A^T A) diag(rq), so only one 8x8 per-pixel matrix reaches
  the big stage-2 contractions. Stage-2 softmax normalizer folds into
  v~ = v * (1/Z_d) and the final v+v doubling is folded into Wv on host.
"""
import sys
import time

sys.path.insert(0, "/opt/trn_rl_repo")

import numpy as np
import ml_dtypes
from contextlib import ExitStack

from concourse import bass
import concourse.mybir as mybir
from concourse.bass_utils import run_bass_kernel_spmd

F32 = mybir.dt.float32
BF16 = mybir.dt.bfloat16
AF = mybir.ActivationFunctionType
MUL = mybir.AluOpType.mult
ADD = mybir.AluOpType.add
AX = mybir.AxisListType.X

NCORES = 8
C = 512
H, D = 8, 64

LAST_EXEC_NS = None
LAST_WALL_NS = None
_CACHE = {}


class _Prog:
    """Static program with automatic RAW/WAR/WAW semaphore insertion.

    Ops are emitted per-engine in list order. Each op declares the buffers it
    reads/writes; dependencies become wait_ge on the producing engine's
    semaphore (every engine's ops inc its own semaphore: compute +1, DMA +16).
    """

    BIG = 1 << 60

    def __init__(self):
        self.ops = []  # dict: engine, fn, deps(set of op ids)
        self.writes = {}  # tensor -> list of (op id, lo, hi)
        self.reads = {}   # tensor -> list of (op id, lo, hi)

    @staticmethod
    def _key(b):
        # "name" (whole tensor) or ("name", lo, hi) element ranges
        if isinstance(b, str):
            return b, 0, _Prog.BIG
        return b

    def op(self, engine, fn, reads=(), writes=(), sem=None):
        deps = set()
        rds = [self._key(b) for b in reads]
        wrs = [self._key(b) for b in writes]
        for name, lo, hi in rds:
            for oid, wlo, whi in self.writes.get(name, ()):
                if wlo < hi and lo < whi:
                    deps.add(oid)
        for name, lo, hi in wrs:
            for oid, rlo, rhi in self.reads.get(name, ()):
                if rlo < hi and lo < rhi:
                    deps.add(oid)
            for oid, wlo, whi in self.writes.get(name, ()):
                if wlo < hi and lo < whi:
                    deps.add(oid)
        i = len(self.ops)
        self.ops.append({"engine": engine, "sem": sem or engine, "fn": fn, "deps": deps})
        for name, lo, hi in rds:
            self.reads.setdefault(name, []).append((i, lo, hi))
        for name, lo, hi in wrs:
            # drop fully-covered older entries to bound list growth
            self.writes[name] = [
                t for t in self.writes.get(name, ()) if not (lo <= t[1] and t[2] <= hi)
            ] + [(i, lo, hi)]
            self.reads[name] = [
                t for t in self.reads.get(name, ()) if not (lo <= t[1] and t[2] <= hi)
            ]
        return i


def build_fused(npx):
    G = npx // 128
    nc = bass.Bass()
    xall = nc.dram_tensor("xall", [128, 4 * npx], BF16, kind="ExternalInput")
    wall = nc.dram_tensor("wall", [128, 16 * 512], BF16, kind="ExternalInput")
    biasin = nc.dram_tensor("biasin", [1, 512], BF16, kind="ExternalInput")
    yout = nc.dram_tensor("yout", [npx, 512], BF16, kind="ExternalOutput")
    scr = nc.dram_tensor("scr", [npx, 512], BF16, kind="Internal")

    with ExitStack() as ctx:
        def sb(name, shp, dt):
            return ctx.enter_context(nc.sbuf_tensor(name, shp, dt))

        xs = sb("xs", [128, 4 * npx], BF16)
        ws = sb("ws", [128, 16 * 512], BF16)
        bias = sb("bias", [1, 512], BF16)
        ones = sb("ones", [1, 128], BF16)
        qbT2 = [sb(f"qbTb{i}", [128, 512], BF16) for i in range(2)]  # [e*8+h]
        kbT2 = [sb(f"kbTb{i}", [128, 512], BF16) for i in range(2)]  # [d*8+h]
        vb2 = [sb(f"vbb{i}", [128, 512], BF16) for i in range(2)]    # [h*64+d]
        zsq = sb("zsq", [128, 1024], BF16)  # q,k squares
        zmid = sb("zmid", [128, 4096], BF16)
        zmid2 = sb("zmid2", [128, 16384], BF16)
        zbig = sb("zbig", [128, 32768], BF16)
        Sf = sb("Sf", [128, 4096], F32)
        est2 = [sb(f"estb{i}", [128, 4096], BF16) for i in range(2)]
        nrm = sb("nrm", [128, 16], F32)     # nq | nk
        rcp = sb("rcp", [128, 16], F32)     # tq | tk
        rqk2 = [sb(f"rqkb{i}", [128, 16], F32) for i in range(2)]
        G64 = sb("G64", [128, 64], F32)
        tv8 = sb("tv8", [128, 8], F32)
        rv8 = sb("rv8", [128, 8], F32)
        rvv = sb("rvv", [128, 64], F32)
        gh = sb("gh", [128, 64], F32)
        eg = sb("eg", [128, 64], F32)
        sa8 = sb("sa8", [128, 8], F32)
        tt8 = sb("tt8", [128, 8], F32)
        w8 = sb("w8", [128, 8], F32)
        t1 = sb("t1", [128, 64], F32)
        zb = sb("zb", [128, 512], F32)
        B64 = sb("B64", [128, 64], F32)
        rkq = sb("rkq", [128, 64], F32)
        btT = sb("btT", [128, 64], BF16)    # [g*8+h]
        k2f = sb("k2f", [128, 512], F32)    # [g*64+d]
        k2T2 = [sb(f"k2Tb{i}", [128, 512], BF16) for i in range(2)]  # [d*8+g]
        Z64 = sb("Z64", [128, 64], F32)
        rz = sb("rz", [128, 64], BF16)
        vt2 = [sb(f"vtb{i}", [128, 512], BF16) for i in range(2)]
        oab2 = [sb(f"oabb{i}", [128, 512], BF16) for i in range(2)]
        xscr = sb("xscr", [128, 4 * npx], BF16)
        yb = sb("yb", [128, 512], BF16)

        psq2 = [ctx.enter_context(nc.psum_tensor(f"psqb{i}", [128, 512], F32)) for i in range(2)]
        psk2 = [ctx.enter_context(nc.psum_tensor(f"pskb{i}", [128, 512], F32)) for i in range(2)]
        psv2 = [ctx.enter_context(nc.psum_tensor(f"psvb{i}", [128, 512], F32)) for i in range(2)]
        psy = ctx.enter_context(nc.psum_tensor("psy", [128, 512], F32))

        sems = {
            k: ctx.enter_context(nc.semaphore(f"sem_{k}"))
            for k in ("t", "a", "v", "p", "si", "sc", "sg", "sy")
        }

        P = _Prog()
        xsr = xs[:].rearrange("p (c n) -> p c n", c=4)
        wsr = ws[:].rearrange("p (w c n) -> p w c n", w=4, c=4)

        # ---- loads ----
        P.op("s", lambda e: e.dma_start(out=xs[:], in_=xall[:]), writes=["xs"], sem="si")
        P.op("s", lambda e: e.dma_start(out=ws[:], in_=wall[:]), writes=["ws"], sem="si")
        P.op("s", lambda e: e.dma_start(out=bias[:], in_=biasin[:]), writes=["bias"], sem="si")
        P.op("p", lambda e: e.memset(ones[:], 1.0), writes=["ones"])

        def mm_qkv(g, wi, ps, psname):
            def fn(e):
                for ci in range(4):
                    mm = e.matmul(
                        ps[:],
                        xsr[:, ci, g * 128 : (g + 1) * 128],
                        wsr[:, wi, ci, :],
                        start=(ci == 0),
                        stop=(ci == 3),
                    )
                return mm
            return fn

        # ---- per-group attention ----
        def emit_group(g):
            par = g % 2
            qbT, kbT, vb = qbT2[par], kbT2[par], vb2[par]
            est, rqk, k2T = est2[par], rqk2[par], k2T2[par]
            vt, oab = vt2[par], oab2[par]
            psq, psk, psv = psq2[par], psk2[par], psv2[par]
            pfx = f"_{par}"
            P.op("t", mm_qkv(g, 0, psq, "psq"), reads=["xs", "ws"], writes=["psq" + pfx])
            P.op("t", mm_qkv(g, 1, psk, "psk"), reads=["xs", "ws"], writes=["psk" + pfx])
            P.op("t", mm_qkv(g, 2, psv, "psv"), reads=["xs", "ws"], writes=["psv" + pfx])

            # psum -> sbuf copies (ACT), with layout transforms
            P.op(
                "a",
                lambda e: e.activation(
                    qbT[:].rearrange("p (e h) -> p h e", h=H), psq[:], AF.Copy
                ),
                reads=["psq" + pfx],
                writes=["qbT" + pfx],
            )
            P.op(
                "a",
                lambda e: e.activation(
                    kbT[:].rearrange("p (d h) -> p h d", h=H), psk[:], AF.Copy
                ),
                reads=["psk" + pfx],
                writes=["kbT" + pfx],
            )
            P.op(
                "a",
                lambda e: e.activation(vb[:], psv[:], AF.Copy),
                reads=["psv" + pfx],
                writes=["vb" + pfx],
            )

            # squares for q,k norms (gpsimd)
            P.op(
                "p",
                lambda e: e.tensor_tensor(zsq[:, 0:512], qbT[:], qbT[:], op=MUL),
                reads=["qbT" + pfx],
                writes=["zsq_q"],
            )
            P.op(
                "p",
                lambda e: e.tensor_tensor(zsq[:, 512:1024], kbT[:], kbT[:], op=MUL),
                reads=["kbT" + pfx],
                writes=["zsq_k"],
            )
            # norm sums: nq over e for each h; zsq_q layout [e*8+h]
            P.op(
                "v",
                lambda e: e.reduce_sum(
                    nrm[:, 0:8],
                    zsq[:, 0:512].rearrange("p (e h) -> p h e", h=H),
                    axis=AX,
                ),
                reads=["zsq_q"],
                writes=["nq"],
            )
            P.op(
                "v",
                lambda e: e.reduce_sum(
                    nrm[:, 8:16],
                    zsq[:, 512:1024].rearrange("p (d h) -> p h d", h=H),
                    axis=AX,
                ),
                reads=["zsq_k"],
                writes=["nk"],
            )
            P.op(
                "v",
                lambda e: e.reciprocal(rcp[:], nrm[:]),
                reads=["nq", "nk"],
                writes=["rcp"],
            )
            P.op(
                "a",
                lambda e: e.activation(rqk[:], rcp[:], AF.Sqrt),
                reads=["rcp"],
                writes=["rqk" + pfx],
            )

            # gram of v: zmid[h,g,d] = vb[h,d]*vb[g,d]
            def gram(e):
                v3 = vb[:].rearrange("p (h d) -> p h d", h=H)
                in0 = v3.unsqueeze(2).broadcast_to([128, H, H, D])
                in1 = v3.unsqueeze(1).broadcast_to([128, H, H, D])
                return e.tensor_tensor(
                    zmid[:].rearrange("p (h g d) -> p h g d", h=H, g=H),
                    in0, in1, op=MUL,
                )
            P.op("v", gram, reads=["vb" + pfx], writes=["zmid"])
            P.op(
                "v",
                lambda e: e.reduce_sum(
                    G64[:],
                    zmid[:].rearrange("p (hg d) -> p hg d", d=D),
                    axis=AX,
                ),
                reads=["zmid"],
                writes=["G64"],
            )
            P.op(
                "v",
                lambda e: e.reciprocal(tv8[:], G64[:, 0 : 64 : H + 1]),
                reads=["G64"],
                writes=["tv8"],
            )
            P.op(
                "a",
                lambda e: e.activation(rv8[:], tv8[:], AF.Sqrt),
                reads=["tv8"],
                writes=["rv8"],
            )
            P.op(
                "v",
                lambda e: e.tensor_tensor(
                    rvv[:].rearrange("p (h g) -> p h g", h=H),
                    rv8[:].unsqueeze(2).broadcast_to([128, H, H]),
                    rv8[:].unsqueeze(1).broadcast_to([128, H, H]),
                    op=MUL,
                ),
                reads=["rv8"],
                writes=["rvv"],
            )
            P.op(
                "v",
                lambda e: e.tensor_tensor(gh[:], G64[:], rvv[:], op=MUL),
                reads=["G64", "rvv"],
                writes=["gh"],
            )
            P.op(
                "a",
                lambda e: e.activation(eg[:], gh[:], AF.Exp),
                reads=["gh"],
                writes=["eg"],
            )
            P.op(
                "v",
                lambda e: e.reduce_sum(
                    sa8[:], eg[:].rearrange("p (f g) -> p f g", f=H), axis=AX
                ),
                reads=["eg"],
                writes=["sa8"],
            )
            P.op(
                "v",
                lambda e: e.reciprocal(tt8[:], sa8[:]),
                reads=["sa8"],
                writes=["tt8"],
            )
            P.op(
                "a",
                lambda e: e.activation(w8[:], tt8[:], AF.Square),
                reads=["tt8"],
                writes=["w8"],
            )
            P.op(
                "v",
                lambda e: e.tensor_tensor(
                    t1[:].rearrange("p (f g) -> p f g", f=H),
                    eg[:].rearrange("p (f g) -> p f g", f=H),
                    w8[:].unsqueeze(2).broadcast_to([128, H, H]),
                    op=MUL,
                ),
                reads=["eg", "w8"],
                writes=["t1"],
            )
            # zb[h,g,f] = t1[f,h] * eg[f,g]
            def zbmul(e):
                t1v = t1[:].rearrange("p (f h) -> p f h", f=H)  # [p,f,h]
                egv = eg[:].rearrange("p (f g) -> p f g", f=H)
                in0 = t1v.transpose([0, 2, 1]).unsqueeze(2).broadcast_to([128, H, H, H])
                in1 = egv.transpose([0, 2, 1]).unsqueeze(1).broadcast_to([128, H, H, H])
                return e.tensor_tensor(
                    zb[:].rearrange("p (h g f) -> p h g f", h=H, g=H), in0, in1, op=MUL
                )
            P.op("v", zbmul, reads=["t1", "eg"], writes=["zb"])
            P.op(
                "v",
                lambda e: e.reduce_sum(
                    B64[:], zb[:].rearrange("p (hg f) -> p hg f", f=H), axis=AX
                ),
                reads=["zb"],
                writes=["B64"],
            )
            # rkq[h,g] = rk[h]*rq[g]
            P.op(
                "v",
                lambda e: e.tensor_tensor(
                    rkq[:].rearrange("p (h g) -> p h g", h=H),
                    rqk[:, 8:16].unsqueeze(2).broadcast_to([128, H, H]),
                    rqk[:, 0:8].unsqueeze(1).broadcast_to([128, H, H]),
                    op=MUL,
                ),
                reads=["rqk" + pfx],
                writes=["rkq"],
            )
            # btT[g*8+h] = B[h,g] * rkq[h,g]
            P.op(
                "v",
                lambda e: e.scalar_tensor_tensor(
                    btT[:].rearrange("p (g h) -> p h g", g=H),
                    B64[:], 1.0, rkq[:], op0=MUL, op1=MUL,
                ),
                reads=["B64", "rkq"],
                writes=["btT"],
            )
            # zk2[g,d,h] = btT[g,h] * kbT[d,h]
            def zk2mul(e):
                b3 = btT[:].rearrange("p (g h) -> p g h", g=H)
                k3 = kbT[:].rearrange("p (d h) -> p d h", d=D)
                in0 = b3.unsqueeze(2).broadcast_to([128, H, D, H])
                in1 = k3.unsqueeze(1).broadcast_to([128, H, D, H])
                return e.tensor_tensor(
                    zmid[:].rearrange("p (g d h) -> p g d h", g=H, d=D),
                    in0, in1, op=MUL,
                )
            P.op("p", zk2mul, reads=["btT", "kbT" + pfx], writes=["zmid"])
            P.op(
                "v",
                lambda e: e.reduce_sum(
                    k2f[:], zmid[:].rearrange("p (gd h) -> p gd h", h=H), axis=AX
                ),
                reads=["zmid"],
                writes=["k2f"],
            )
            # k2T[d*8+g] = k2f[g*64+d] (cast+transpose)
            P.op(
                "a",
                lambda e: e.activation(
                    k2T[:].rearrange("p (d g) -> p g d", d=D),
                    k2f[:], AF.Copy,
                ),
                reads=["k2f"],
                writes=["k2T" + pfx],
            )
            # zS[d,e,g] = k2T[d,g] * qbT[e,g]  -- split by d across V (0:36) / P (36:64)
            DV = 36  # V's d-share
            def zsmul_part(d0, d1):
                def fn(e):
                    k3 = k2T[:].rearrange("p (d g) -> p d g", d=D)[:, d0:d1, :]
                    q3 = qbT[:].rearrange("p (e g) -> p e g", e=D)
                    nd = d1 - d0
                    in0 = k3.unsqueeze(2).broadcast_to([128, nd, D, H])
                    in1 = q3.unsqueeze(1).broadcast_to([128, nd, D, H])
                    out = zbig[:, d0 * 512 : d1 * 512].rearrange(
                        "p (d e g) -> p d e g", e=D, g=H
                    )
                    return e.tensor_tensor(out, in0, in1, op=MUL)
                return fn
            P.op("v", zsmul_part(0, DV), reads=["k2T" + pfx, "qbT" + pfx], writes=[("zbig", 0, 18432)])
            P.op("p", zsmul_part(DV, D), reads=["k2T" + pfx, "qbT" + pfx], writes=[("zbig", 18432, 32768)])

            # S-tree: half-pair adds over g (unit-stride => DVE 2x), V/P parts
            def hp(src_ap, dst_ap, k):
                # src viewed [p, x, k] -> dst [p, x, k//2] adding halves
                def fn(e):
                    s3 = src_ap.rearrange("p (x k) -> p x k", k=k)
                    d3 = dst_ap.rearrange("p (x k) -> p x k", k=k // 2)
                    return e.tensor_tensor(
                        d3, s3[:, :, 0 : k // 2], s3[:, :, k // 2 : k], op=ADD
                    )
                return fn
            DE1 = DV * 64  # 2304
            P.op("v", hp(zbig[:, 0 : DE1 * 8], zmid2[:, 0 : DE1 * 4], 8),
                 reads=[("zbig", 0, 18432)], writes=[("zmid2", 0, 9216)])
            P.op("p", hp(zbig[:, DE1 * 8 : 32768], zmid2[:, DE1 * 4 : 16384], 8),
                 reads=[("zbig", 18432, 32768)], writes=[("zmid2", 9216, 16384)])
            P.op("v", hp(zmid2[:, 0 : DE1 * 4], zbig[:, 0 : DE1 * 2], 4),
                 reads=[("zmid2", 0, 9216)], writes=[("zbig", 0, 4608)])
            P.op("p", hp(zmid2[:, DE1 * 4 : 16384], zbig[:, DE1 * 2 : 8192], 4),
                 reads=[("zmid2", 9216, 16384)], writes=[("zbig", 4608, 8192)])
            P.op("v", hp(zbig[:, 0 : DE1 * 2], Sf[:, 0:DE1], 2),
                 reads=[("zbig", 0, 4608)], writes=[("Sf", 0, 2304)])
            P.op("p", hp(zbig[:, DE1 * 2 : 8192], Sf[:, DE1:4096], 2),
                 reads=[("zbig", 4608, 8192)], writes=[("Sf", 2304, 4096)])

            # expS transposed: est[e*64+d] = exp(S[d*64+e])
            P.op(
                "a",
                lambda e: e.activation(
                    est[:].rearrange("p (e d) -> p d e", e=D), Sf[:], AF.Exp
                ),
                reads=[("Sf", 0, 4096)],
                writes=["est" + pfx],
            )
            # Z[d] = sum_e expS[d,e]
            P.op(
                "v",
                lambda e: e.reduce_sum(
                    Z64[:], est[:].rearrange("p (e d) -> p d e", e=D), axis=AX
                ),
                reads=["est" + pfx],
                writes=["Z64"],
            )
            def rzrecip(e):
                with nc.allow_low_precision(reason="1/Z at bf16 is within budget"):
                    return e.reciprocal(rz[:], Z64[:])
            P.op("v", rzrecip, reads=["Z64"], writes=["rz"])
            # vt[h,d] = vb[h,d]*rz[d]
            P.op(
                "v",
                lambda e: e.tensor_tensor(
                    vt[:].rearrange("p (h d) -> p h d", h=H),
                    vb[:].rearrange("p (h d) -> p h d", h=H),
                    rz[:].unsqueeze(1).broadcast_to([128, H, D]),
                    op=MUL,
                ),
                reads=["vb" + pfx, "rz"],
                writes=["vt" + pfx],
            )
            # zO[h,e,d] = vt[h,d]*est[e,d]  -- split by h across V (0:5) / P (5:8)
            HV = 5
            def zomul_part(h0, h1):
                def fn(e):
                    v3 = vt[:].rearrange("p (h d) -> p h d", h=H)[:, h0:h1, :]
                    e3 = est[:].rearrange("p (e d) -> p e d", e=D)
                    nh = h1 - h0
                    in0 = v3.unsqueeze(2).broadcast_to([128, nh, D, D])
                    in1 = e3.unsqueeze(1).broadcast_to([128, nh, D, D])
                    out = zbig[:, h0 * 4096 : h1 * 4096].rearrange(
                        "p (h e d) -> p h e d", e=D, d=D
                    )
                    return e.tensor_tensor(out, in0, in1, op=MUL)
                return fn
            P.op("v", zomul_part(0, HV),
                 reads=["vt" + pfx, "est" + pfx], writes=[("zbig", 0, 20480)])
            P.op("p", zomul_part(HV, H),
                 reads=["vt" + pfx, "est" + pfx], writes=[("zbig", 20480, 32768)])

            # O-tree: half-pair adds over d, V/P parts (he-aligned)
            HE1 = HV * 64  # 320
            P.op("v", hp(zbig[:, 0 : HE1 * 64], zmid2[:, 0 : HE1 * 32], 64),
                 reads=[("zbig", 0, 20480)], writes=[("zmid2", 0, 10240)])
            P.op("p", hp(zbig[:, HE1 * 64 : 32768], zmid2[:, HE1 * 32 : 16384], 64),
                 reads=[("zbig", 20480, 32768)], writes=[("zmid2", 10240, 16384)])
            P.op("v", hp(zmid2[:, 0 : HE1 * 32], zbig[:, 0 : HE1 * 16], 32),
                 reads=[("zmid2", 0, 10240)], writes=[("zbig", 0, 5120)])
            P.op("p", hp(zmid2[:, HE1 * 32 : 16384], zbig[:, HE1 * 16 : 8192], 32),
                 reads=[("zmid2", 10240, 16384)], writes=[("zbig", 5120, 8192)])
            P.op("v", hp(zbig[:, 0 : HE1 * 16], zmid[:, 0 : HE1 * 8], 16),
                 reads=[("zbig", 0, 5120)], writes=[("zmid", 0, 2560)])
            P.op("p", hp(zbig[:, HE1 * 16 : 8192], zmid[:, HE1 * 8 : 4096], 16),
                 reads=[("zbig", 5120, 8192)], writes=[("zmid", 2560, 4096)])
            P.op("v", hp(zmid[:, 0 : HE1 * 8], zmid2[:, 0 : HE1 * 4], 8),
                 reads=[("zmid", 0, 2560)], writes=[("zmid2", 0, 1280)])
            P.op("p", hp(zmid[:, HE1 * 8 : 4096], zmid2[:, HE1 * 4 : 2048], 8),
                 reads=[("zmid", 2560, 4096)], writes=[("zmid2", 1280, 2048)])
            P.op("v", hp(zmid2[:, 0 : HE1 * 4], zmid[:, 0 : HE1 * 2], 4),
                 reads=[("zmid2", 0, 1280)], writes=[("zmid", 0, 640)])
            P.op("p", hp(zmid2[:, HE1 * 4 : 2048], zmid[:, HE1 * 2 : 1024], 4),
                 reads=[("zmid2", 1280, 2048)], writes=[("zmid", 640, 1024)])
            P.op("v", hp(zmid[:, 0 : HE1 * 2], oab[:, 0:HE1], 2),
                 reads=[("zmid", 0, 640)], writes=[("oab" + pfx, 0, 320)])
            P.op("p", hp(zmid[:, HE1 * 2 : 1024], oab[:, HE1:512], 2),
                 reads=[("zmid", 640, 1024)], writes=[("oab" + pfx, 320, 512)])

            # store attention output rows to DRAM scratch
            def store_scr(g):
                def fn(e):
                    return e.dma_start(
                        out=scr[g * 128 : (g + 1) * 128, :], in_=oab[:]
                    )
                return fn
            P.op("s", store_scr(g), reads=["oab" + pfx], writes=[f"scr{g}", "scr_order"], sem="sc")

        for g in range(G):
            emit_group(g)

        # ---- scramble gather: xscr blocks ----
        # xscrT[c'=pm*8+h, r'=pb*64+e] = scr[p=pb*64+pm, h*64+e]
        def gather(e):
            src_r = scr[:].rearrange("(pb pm) (h e) -> pm pb h e", pm=64, h=H)
            last = None
            for Bi in range(4):
                xb = xscr[:, Bi * npx : (Bi + 1) * npx]
                for pmq in range(16):
                    pm = 16 * Bi + pmq
                    dst = xb[8 * pmq : 8 * pmq + 8, :].rearrange(
                        "h (pb e) -> h pb e", e=D
                    )
                    s_ap = src_r[pm, :, :, :].transpose([1, 0, 2])
                    last = e.dma_start(out=dst, in_=s_ap)
            return last
        gid = P.op(
            "s", gather,
            reads=[f"scr{g}" for g in range(G)],
            writes=["xscr"],
            sem="sg",
        )
        # gather emits 64 DMAs but op framework incs once; fix: account below.
        _GATHER_DMAS = 64

        # ---- proj ----
        for rg in range(G):
            def mm_proj(rg):
                def fn(e):
                    for ci in range(4):
                        e.matmul(
                            psy[:],
                            xscr[:, ci * npx + rg * 128 : ci * npx + (rg + 1) * 128],
                            wsr[:, 3, ci, :],
                            start=(ci == 0),
                            stop=False,
                        )
                    mm = e.matmul(
                        psy[:], ones[:], bias[:], start=False, stop=True
                    )
                    return mm
                return fn
            P.op("t", mm_proj(rg), reads=["xscr", "ws", "ones", "bias"],
                 writes=["psy"])
            P.op(
                "a",
                lambda e: e.activation(yb[:], psy[:], AF.Copy),
                reads=["psy"],
                writes=["yb"],
            )
            def store_y(rg):
                def fn(e):
                    return e.dma_start(
                        out=yout[rg * 128 : (rg + 1) * 128, :], in_=yb[:]
                    )
                return fn
            P.op("s", store_y(rg), reads=["yb"], writes=[f"yout{rg}"], sem="sy")

        # ---- emit ----
        # fix gather op inc accounting: it emits 64 dma_starts, each must inc;
        # we gave it one inc. Simplest: make each dma in gather inc and adjust
        # the cumulative count. Easier: treat gather as 64 increments.
        run_engine = _emit_prog(P, nc, sems, gid, _GATHER_DMAS)

        with nc.Block() as block:
            @block.sync
            def _(eng):
                run_engine("s", eng)

            @block.tensor
            def _(eng):
                run_engine("t", eng)

            @block.scalar
            def _(eng):
                run_engine("a", eng)

            @block.vector
            def _(eng):
                run_engine("v", eng)

            @block.gpsimd
            def _(eng):
                run_engine("p", eng)

    return nc


DMA_SEMS = ("si", "sc", "sg", "sy")
# sems whose waiters must always wait for the sem's running total at that
# point (their DMAs complete out of order):
TOTAL_SEMS = ("si", "sg")


def _emit_prog(P, nc, sems, gather_id, gather_n):
    """Emit P's ops; the gather op emits gather_n DMAs, each inc'ing by 16."""
    cum = {k: 0 for k in sems}
    val = [0] * len(P.ops)
    for i, o in enumerate(P.ops):
        k = o["sem"]
        if i == gather_id:
            cum[k] += 16 * gather_n
        else:
            cum[k] += 16 if k in DMA_SEMS else 1
        val[i] = cum[k]
    by_engine = {}
    for i, o in enumerate(P.ops):
        by_engine.setdefault(o["engine"], []).append(i)

    def run_engine(ename, eng):
        watermark = {}
        for i in by_engine.get(ename, ()):
            o = P.ops[i]
            need = {}
            for d in o["deps"]:
                dk = P.ops[d]["sem"]
                v = cum[dk] if dk in TOTAL_SEMS else val[d]
                need[dk] = max(need.get(dk, 0), v)
            for dk, v in sorted(need.items()):
                if watermark.get(dk, 0) >= v:
                    continue
                eng.wait_ge(sems[dk], v)
                watermark[dk] = v
            k = o["sem"]
            if i == gather_id:
                collected = []
                orig = eng.dma_start

                def wrapped(*a, **kw):
                    ins = orig(*a, **kw)
                    collected.append(ins)
                    return ins

                eng.dma_start = wrapped
                try:
                    o["fn"](eng)
                finally:
                    eng.dma_start = orig
                for ins in collected:
                    ins.then_inc(sems[k], 16)
            else:
                last = o["fn"](eng)
                last.then_inc(sems[k], 16 if k in DMA_SEMS else 1)

    return run_engine


def _build_warmup(npx):
    """Trivial kernel with the fused kernel's exact I/O shapes; run once to
    warm the axon/PJRT/jit infrastructure before the timed launch."""
    nc = bass.Bass()
    xall = nc.dram_tensor("xall", [128, 4 * npx], BF16, kind="ExternalInput")
    wall = nc.dram_tensor("wall", [128, 16 * 512], BF16, kind="ExternalInput")
    biasin = nc.dram_tensor("biasin", [1, 512], BF16, kind="ExternalInput")
    yout = nc.dram_tensor("yout", [npx, 512], BF16, kind="ExternalOutput")
    with nc.sbuf_tensor("t", [128, 512], BF16) as t, nc.semaphore("s") as s, nc.Block() as block:
        @block.sync
        def _(sync):
            sync.dma_start(out=t[:], in_=xall[:, 0:512]).then_inc(s, 16)
            sync.wait_ge(s, 16)
            for r in range(npx // 128):
                sync.dma_start(
                    out=yout[r * 128 : (r + 1) * 128, :], in_=t[:]
                ).then_inc(s, 16)
            sync.wait_ge(s, 16 * (1 + npx // 128))
    return nc


def _pack_x(Xc):
    """[npx, 512] -> [128, 4*npx] bf16: out[p, ci, px] = X[px, ci*128+p]"""
    npx = Xc.shape[0]
    xt = Xc.T.reshape(4, 128, npx).transpose(1, 0, 2).reshape(128, 4 * npx)
    return np.ascontiguousarray(xt.astype(ml_dtypes.bfloat16))


def _pack_w(Wq, Wk, Wv2, Wp):
    """4x [512,512] -> [128, 4w*4ci*512co] bf16: out[p,w,ci,co]=W[co,ci*128+p]"""
    ws = np.stack(
        [W.T.reshape(4, 128, 512).transpose(1, 0, 2) for W in (Wq, Wk, Wv2, Wp)],
        axis=1,
    )  # [128, 4w, 4ci, 512]
    return np.ascontiguousarray(
        ws.reshape(128, 16 * 512).astype(ml_dtypes.bfloat16)
    )


def _can_trace():
    """exec_time_ns needs NTFF profiling: native /dev/neuron runs support it
    directly; axon runs need the antenv NTFF hook (probe to avoid a crash on
    builds that lack it)."""
    try:
        from concourse.bass_utils import axon_active
        if not axon_active():
            return True
    except Exception:
        pass
    try:
        from antenv.axon_hooks import get_axon_ntff_profile_hook
    except Exception:
        return False
    try:
        return get_axon_ntff_profile_hook() is not None
    except Exception:
        return False


def kernel(x, Wq, Wk, Wv, conv_w, proj_w, proj_b):
    global LAST_EXEC_NS, LAST_WALL_NS
    x = np.asarray(x, np.float32)
    b, h, w, c = x.shape
    n = h * w
    N = b * n
    npx = N // NCORES
    X = x.reshape(N, c)

    if "fused" not in _CACHE:
        _CACHE["fused"] = build_fused(npx)
        _CACHE["warm"] = _build_warmup(npx)
    nc = _CACHE["fused"]

    wallp = _pack_w(
        np.asarray(Wq, np.float32),
        np.asarray(Wk, np.float32),
        2.0 * np.asarray(Wv, np.float32),
        np.asarray(proj_w, np.float32),
    )
    biasp = np.ascontiguousarray(
        np.asarray(proj_b, np.float32).reshape(1, 512).astype(ml_dtypes.bfloat16)
    )
    in_maps = [
        {
            "xall": _pack_x(X[j * npx : (j + 1) * npx]),
            "wall": wallp,
            "biasin": biasp,
        }
        for j in range(NCORES)
    ]

    # one-time infra warmup (axon/PJRT/jit init), not part of the kernel run
    if "warmed" not in _CACHE:
        win = [
            {
                "xall": np.zeros((128, 4 * npx), ml_dtypes.bfloat16),
                "wall": np.zeros((128, 16 * 512), ml_dtypes.bfloat16),
                "biasin": np.zeros((1, 512), ml_dtypes.bfloat16),
            }
            for _ in range(NCORES)
        ]
        run_bass_kernel_spmd(_CACHE["warm"], win, list(range(NCORES)))
        _CACHE["warmed"] = True

    t0 = time.perf_counter_ns()
    res = None
    if _can_trace():
        try:
            res = run_bass_kernel_spmd(
                nc, in_maps, list(range(NCORES)), trace=True
            )
        except Exception:
            res = None
    if res is None:
        t0 = time.perf_counter_ns()
        res = run_bass_kernel_spmd(nc, in_maps, list(range(NCORES)))
    wall_ns = time.perf_counter_ns() - t0

    LAST_EXEC_NS = res.exec_time_ns
    LAST_WALL_NS = wall_ns

    # unscramble: core j row r' -> y[b=j//2, n' = e*64 + (j%2)*32 + pb]
    y = np.zeros((b, n, c), np.float32)
    rp = np.arange(npx)
    pb2 = rp // 64
    e = rp % 64
    for j in range(NCORES):
        yc = res.results[j]["yout"].astype(np.float32)
        nprime = e * 64 + (j % 2) * 32 + pb2
        y[j // 2, nprime] = yc
    return y.reshape(b, h, w, c)
